# revision 1
# baseline (speedup 1.0000x reference)
"""Hopfield neuron update kernel for 8 Trainium2 NeuronCores.

Computes, for W [N,N], s [N] (+-1), b [N]:
    act       = W @ s - diag(W)*s + (N-1)*b
    new_state = where(act >= 0, 1, -1)

Sharding: row-shard W across 8 cores (each core owns N/8=2048 rows of W,
bias and output), replicate s. Per core the matvec is computed on the
Vector engine with fused multiply+reduce (tensor_tensor_reduce) over
natural-layout W tiles [128 rows x FD cols] streamed from HBM, with the
replicated state vector broadcast across all 128 SBUF partitions once at
startup (GPSIMD partition_broadcast; no HBM traffic). This is memory-bound:
one fp32 pass over W at ~HBM rate.
"""

import os
import sys

import numpy as np

for _p in ("/opt/trn_rl_repo", "/root/.axon_site/_ro/trn_rl_repo"):
    if os.path.isdir(_p) and _p not in sys.path:
        sys.path.insert(0, _p)

N = 16384
NCORES = 8
R = N // NCORES          # rows per core: 2048
P = 128                  # SBUF partitions
G = R // P               # row groups per core: 16
FD = 2048                # DMA tile free size (8 KiB/partition, contiguous)
FC = 1024                # compute chunk size (precision/overhead balance)
SUBS = FD // FC          # compute chunks per DMA tile: 2
NCHUNK = N // FC         # accumulation slots per row group: 16
WBUFS = 14               # in-flight W tiles (DMA prefetch depth)

_CACHE = {}


def _build_nc():
    import concourse.bacc as bacc
    import concourse.mybir as mybir
    from concourse.tile import TileContext

    f32 = mybir.dt.float32
    nc = bacc.Bacc()

    w = nc.dram_tensor("w", [R, N], f32, kind="ExternalInput")
    s = nc.dram_tensor("s", [N], f32, kind="ExternalInput")
    diag_t = nc.dram_tensor("diag_t", [P, G], f32, kind="ExternalInput")
    srows_t = nc.dram_tensor("srows_t", [P, G], f32, kind="ExternalInput")
    bias_t = nc.dram_tensor("bias_t", [P, G], f32, kind="ExternalInput")
    act_o = nc.dram_tensor("act_o", [P, G], f32, kind="ExternalOutput")
    ns_o = nc.dram_tensor("ns_o", [P, G], f32, kind="ExternalOutput")

    with TileContext(nc) as tc:
        with (
            tc.tile_pool(name="consts", bufs=1) as consts,
            tc.tile_pool(name="wpool", bufs=WBUFS) as wpool,
        ):
            sb = consts.tile([P, N], f32)
            partials = consts.tile([P, G * NCHUNK], f32)
            act_acc = consts.tile([P, G], f32)
            dummy = consts.tile([P, 1], f32)

            # Broadcast s across all 128 partitions straight from DRAM with a
            # stride-0 partition access pattern, chunked so compute on chunk c
            # only waits for its own DMA and chunks spread across DMA queues.
            for cd in range(N // FD):
                js = slice(cd * FD, (cd + 1) * FD)
                nc.sync.dma_start(
                    out=sb[:, js], in_=s[None, js].broadcast_to([P, FD])
                )

            for g in range(G):
                rows = slice(g * P, (g + 1) * P)
                for cd in range(N // FD):
                    js = slice(cd * FD, (cd + 1) * FD)
                    wt = wpool.tile([P, FD], f32)
                    nc.sync.dma_start(out=wt[:], in_=w[rows, js])
                    for sub in range(SUBS):
                        ks = slice(sub * FC, (sub + 1) * FC)
                        jc = slice(cd * FD + sub * FC, cd * FD + (sub + 1) * FC)
                        slot = g * NCHUNK + cd * SUBS + sub
                        nc.vector.scalar_tensor_tensor(
                            out=dummy[:].broadcast_to([P, FC]),
                            in0=wt[:, ks],
                            scalar=1.0,
                            in1=sb[:, jc],
                            op0=mybir.AluOpType.bypass,
                            op1=mybir.AluOpType.mult,
                            accum_out=partials[:, slot : slot + 1],
                        )
                nc.vector.tensor_reduce(
                    out=act_acc[:, g : g + 1],
                    in_=partials[:, g * NCHUNK : (g + 1) * NCHUNK],
                    axis=mybir.AxisListType.X,
                    op=mybir.AluOpType.add,
                )

            # Epilogue: act = acc - diag*s_rows + (N-1)*bias; ns = sign(act)
            dt_t = consts.tile([P, G], f32)
            st_t = consts.tile([P, G], f32)
            bt_t = consts.tile([P, G], f32)
            t0 = consts.tile([P, G], f32)
            act_f = consts.tile([P, G], f32)
            ns0 = consts.tile([P, G], f32)
            ns1 = consts.tile([P, G], f32)
            nc.sync.dma_start(out=dt_t[:], in_=diag_t[:, :])
            nc.sync.dma_start(out=st_t[:], in_=srows_t[:, :])
            nc.sync.dma_start(out=bt_t[:], in_=bias_t[:, :])
            nc.vector.tensor_tensor(
                out=t0[:], in0=dt_t[:], in1=st_t[:], op=mybir.AluOpType.mult
            )
            nc.vector.tensor_tensor(
                out=t0[:], in0=act_acc[:], in1=t0[:], op=mybir.AluOpType.subtract
            )
            nc.vector.scalar_tensor_tensor(
                out=act_f[:],
                in0=bt_t[:],
                scalar=float(N - 1),
                in1=t0[:],
                op0=mybir.AluOpType.mult,
                op1=mybir.AluOpType.add,
            )
            nc.vector.tensor_scalar(
                out=ns0[:],
                in0=act_f[:],
                scalar1=0.0,
                scalar2=None,
                op0=mybir.AluOpType.is_ge,
            )
            nc.vector.tensor_scalar(
                out=ns1[:],
                in0=ns0[:],
                scalar1=2.0,
                scalar2=-1.0,
                op0=mybir.AluOpType.mult,
                op1=mybir.AluOpType.add,
            )
            nc.sync.dma_start(out=act_o[:, :], in_=act_f[:])
            nc.sync.dma_start(out=ns_o[:, :], in_=ns1[:])

    nc.finalize()
    return nc


def get_nc():
    if "nc" not in _CACHE:
        _CACHE["nc"] = _build_nc()
    return _CACHE["nc"]


def make_in_maps(weights, state, bias):
    weights = np.ascontiguousarray(weights, dtype=np.float32)
    state = np.ascontiguousarray(state, dtype=np.float32)
    bias = np.ascontiguousarray(bias, dtype=np.float32)
    diag = np.ascontiguousarray(np.diagonal(weights))
    in_maps = []
    for c in range(NCORES):
        rows = slice(c * R, (c + 1) * R)
        in_maps.append(
            {
                "w": weights[rows],
                "s": state,
                "diag_t": np.ascontiguousarray(diag[rows].reshape(G, P).T),
                "srows_t": np.ascontiguousarray(state[rows].reshape(G, P).T),
                "bias_t": np.ascontiguousarray(bias[rows].reshape(G, P).T),
            }
        )
    return in_maps


def gather(results):
    act = np.concatenate([r["act_o"].T.reshape(R) for r in results])
    ns = np.concatenate([r["ns_o"].T.reshape(R) for r in results])
    return act.astype(np.float32), ns.astype(np.float32)


def kernel(weights, state, bias):
    from concourse.bass_utils import run_bass_kernel_spmd

    nc = get_nc()
    in_maps = make_in_maps(weights, state, bias)
    res = run_bass_kernel_spmd(nc, in_maps, list(range(NCORES)))
    return gather(res.results)

